# revision 2
# baseline (speedup 1.0000x reference)
"""Dynamic 3x3 per-pixel filter (DynamicFilterLayer2D) on 8 Trainium2 cores.

Reference: out[b,c,h,w] = sum_{i,j in 3x3} xpad[b,c,h+i,w+j] * f[b,c,(3i+j),h,w]

Sharding: H is split into 8 bands of 32 rows; each core processes all
(b, c) images for its band (data parallel, 1-row halo). Per-core layout:
partitions = 128 (b,c) images (2 groups of 128), free dim = flat pixels.

Compute: a custom DVE op `scan(ADD, Src0*Src1)` streams [pixel, j-tap]
pairs — x via an overlapping access pattern, filters host-interleaved to
[..., w, j] — producing a running sum of products; per-pixel 3-tap sums
are recovered by differencing the running sum at stride 3. Three such
scans (one per i row-tap) are combined with two adds, and one subtract
produces the output. Filter border columns (the taps that multiply
x-padding) are zeroed host-side, so x tiles need no column padding and
all access patterns have uniform strides.
"""

import numpy as np

B, C, H, W = 8, 32, 256, 256
K = 3
N_CORES = 8
BAND = H // N_CORES            # 32 rows per core
R = 4                          # output rows per compute sub-tile
RD = 8                         # rows per DMA super-tile
SUBS = RD // R                 # compute sub-tiles per super-tile (2)
N_SUPERS = BAND // RD          # 4
N_IMG = B * C                  # 256 images
P = 128
N_IMG_GROUPS = N_IMG // P      # 2
F = R * W                      # pixels per partition per sub-tile (1024)
FD = RD * W                    # pixels per partition per super-tile (2048)
X_SUPER = FD + 2 * W + 2       # x elements per super-tile (guards incl)
X_FLAT = (BAND + 2) * W + 2    # per-image padded x row storage

_CACHE = {}


def _register_mac_scan():
    from concourse import dve_ops
    from concourse.dve_ops import DveOp
    from concourse.dve_spec import Spec, Src0, Src1, scan, AluOp, lower
    from concourse.dve_uop import DveOpSpec

    name = "ANT_MAC_SCAN"
    for op in dve_ops.OPS:
        if op.name == name:
            return op

    def _ref(in0, in1, s0, s1, imm2):
        p = np.asarray(in0, np.float32) * np.asarray(in1, np.float32)
        flat = p.reshape(p.shape[0], -1)
        return np.cumsum(flat, axis=1, dtype=np.float32).reshape(p.shape)

    spec = Spec(body=scan(AluOp.ADD, Src0 * Src1), reference=_ref)
    op = DveOp(name, spec, False, {})
    dve_ops.OPS.append(op)
    dve_ops.CUSTOM_DVE_SPECS[name] = spec
    dve_ops._SUB_OPCODE_FOR_NAME[name] = (
        dve_ops._CUSTOM_DVE_ROW_BASE + len(dve_ops.OPS) - 1)
    for ver in ("v3", "v4"):
        dve_ops._COMPILE_CACHE[(name, ver)] = DveOpSpec(
            name=name,
            opcode=dve_ops.get_dve_sub_opcode(name),
            uops=lower(spec, ver=ver),
            rd1_en=True,
        )
    return op


def _strided_ap(tile_ap, dims, offset):
    """Copy of tile_ap with free dims replaced by [[step, count], ...]
    (element units) at element offset; partition dim preserved."""
    import bass_rust
    c = tile_ap.copy()
    part = list(c.ap)[0]
    c.ap = bass_rust.VecI64Pair([list(part)] + [list(d) for d in dims])
    c.offset = offset
    return c


def _build_module():
    import concourse.bacc as bacc
    import concourse.mybir as mybir
    from concourse.tile import TileContext

    mac_scan = _register_mac_scan()
    fp32 = mybir.dt.float32
    add = mybir.AluOpType.add
    sub = mybir.AluOpType.subtract

    nc = bacc.Bacc("TRN2", target_bir_lowering=False, debug=False)
    x_d = nc.dram_tensor("x_s", [N_IMG, X_FLAT], fp32,
                         kind="ExternalInput").ap()
    # host-interleaved filters: [img, i, band_row, w, j]
    f_d = nc.dram_tensor("f_s", [N_IMG, K, BAND, W, K], fp32,
                         kind="ExternalInput").ap()
    o_d = nc.dram_tensor("o_s", [N_IMG, BAND, W], fp32,
                         kind="ExternalOutput").ap()

    with TileContext(nc) as tc:
        with (
            tc.tile_pool(name="xp", bufs=2) as xpool,
            tc.tile_pool(name="fp", bufs=3) as fpool,
            tc.tile_pool(name="s0p", bufs=1) as s0pool,
            tc.tile_pool(name="s1p", bufs=1) as s1pool,
            tc.tile_pool(name="s2p", bufs=1) as s2pool,
            tc.tile_pool(name="vp", bufs=2) as vpool,
            tc.tile_pool(name="op", bufs=2) as opool,
        ):
            scpools = [s0pool, s1pool, s2pool]
            # per image-group list of (row_start, rows) super-tiles; the
            # schedule ends with two half supers so the compute backlog
            # after the final filter bytes arrive is halved
            supers = {
                g: [(t2 * RD, RD) for t2 in range(N_SUPERS)]
                for g in range(N_IMG_GROUPS)
            }
            supers[N_IMG_GROUPS - 1] = (
                [(t2 * RD, RD) for t2 in range(N_SUPERS - 1)]
                + [(BAND - RD, R), (BAND - R, 2), (BAND - 2, 1), (BAND - 1, 1)]
            )
            for g in range(N_IMG_GROUPS):
                for (r0, rd) in supers[g]:
                    p0 = g * P
                    fd = rd * W
                    n_subs = rd // R
                    xt = xpool.tile([P, X_SUPER], fp32, tag="x")
                    nc.gpsimd.dma_start(
                        out=xt[:, 0:fd + 2 * W + 2],
                        in_=x_d[p0:p0 + P, r0 * W: r0 * W + fd + 2 * W + 2],
                    )
                    fts = []
                    for i in range(K):
                        ft = fpool.tile([P, K * FD], fp32, tag="f", name="ft")
                        nc.sync.dma_start(
                            out=ft[:, 0:K * fd],
                            in_=f_d[p0:p0 + P, i, r0: r0 + rd, :, :],
                        )
                        fts.append(ft)
                    ot = opool.tile([P, FD], fp32, tag="o")
                    subs_list = []
                    sr = 0
                    while sr < rd:
                        rr = min(R, rd - sr)
                        subs_list.append((sr, rr))
                        sr += rr
                    for (sr, rr) in subs_list:
                        fs = rr * W       # pixels in this sub-tile
                        ps = sr * W       # local pixel start within super
                        vt = vpool.tile([P, F + 1], fp32, tag="v", name="vt")
                        nc.gpsimd.memset(vt[:, 0:1], 0.0)
                        scs = []
                        for i in range(K):
                            sct = scpools[i].tile([P, K * F], fp32,
                                                  tag=f"sc{i}", name="sct")
                            in0 = _strided_ap(xt[:, :], [[1, fs], [1, K]],
                                              ps + i * W)
                            in1 = _strided_ap(fts[i][:, :], [[K, fs], [1, K]],
                                              ps * K)
                            sc_out = _strided_ap(sct[:, :], [[K, fs], [1, K]],
                                                 0)
                            nc.vector._custom_dve(mac_scan, out=sc_out,
                                                  in0=in0, in1=in1)
                            scs.append(sct)
                        A = [_strided_ap(scs[i][:, :], [[K, fs]], K - 1)
                             for i in range(K)]
                        nc.vector.tensor_tensor(vt[:, 1:fs + 1], A[0], A[1],
                                                add)
                        nc.vector.tensor_tensor(vt[:, 1:fs + 1],
                                                vt[:, 1:fs + 1], A[2], add)
                        nc.vector.tensor_tensor(ot[:, ps:ps + fs],
                                                vt[:, 1:fs + 1], vt[:, 0:fs],
                                                sub)
                    # last (small) supers: HWDGE out skips the Q7 descriptor
                    # hop on the end-of-kernel critical chain
                    out_eng = nc.sync if rd < RD else nc.gpsimd
                    out_eng.dma_start(
                        out=o_d[p0:p0 + P, r0:r0 + rd, :],
                        in_=ot[:, 0:fd],
                    )
    nc.compile()
    return nc


def _get_module():
    if "nc" not in _CACHE:
        _CACHE["nc"] = _build_module()
    return _CACHE["nc"]


def _shard_inputs(x, dynamic_filters):
    """Per-core input maps. x: [B,C,H,W] f32, filters: [B,C*9,H,W] f32."""
    xp = np.pad(x, ((0, 0), (0, 0), (1, 1), (0, 0)))   # pad rows only
    # filters -> [B, C, i, j, H, W] -> zero border cols -> [img, i, H, W, j]
    f6 = dynamic_filters.reshape(B, C, K, K, H, W).copy()
    f6[:, :, :, 0, :, 0] = 0.0      # j=0 taps multiply x col -1
    f6[:, :, :, 2, :, W - 1] = 0.0  # j=2 taps multiply x col W
    f_int = np.ascontiguousarray(
        f6.transpose(0, 1, 2, 4, 5, 3)).reshape(N_IMG, K, H, W, K)

    in_maps = []
    for n in range(N_CORES):
        r = n * BAND
        xs = xp[:, :, r:r + BAND + 2, :].reshape(N_IMG, (BAND + 2) * W)
        xs_flat = np.zeros((N_IMG, X_FLAT), np.float32)
        xs_flat[:, 1:-1] = xs
        fs = np.ascontiguousarray(f_int[:, :, r:r + BAND])
        in_maps.append({"x_s": xs_flat, "f_s": fs})
    return in_maps


def kernel(x, dynamic_filters, _trace=False):
    from concourse import bass_utils

    x = np.asarray(x, dtype=np.float32)
    dynamic_filters = np.asarray(dynamic_filters, dtype=np.float32)
    nc = _get_module()
    in_maps = _shard_inputs(x, dynamic_filters)
    res = bass_utils.run_bass_kernel_spmd(
        nc, in_maps, list(range(N_CORES)), trace=_trace)
    out = np.concatenate(
        [res.results[n]["o_s"].reshape(B, C, BAND, W) for n in range(N_CORES)],
        axis=2)
    _CACHE["last_exec_time_ns"] = res.exec_time_ns
    if res.instructions_and_trace is not None:
        _CACHE["trace_path"] = res.instructions_and_trace[1]
    return out



# revision 3
# speedup vs baseline: 1.5523x; 1.5523x over previous
"""Dynamic 3x3 per-pixel filter (DynamicFilterLayer2D) on 8 Trainium2 cores.

Reference: out[b,c,h,w] = sum_{i,j in 3x3} xpad[b,c,h+i,w+j] * f[b,c,(3i+j),h,w]

Sharding: H split into 8 bands of 32 rows per core (data parallel, 1-row
halo). Per-core layout: partitions = 128 (b,c) images (2 groups), free dim
= flat pixels.

Compute (per 8-row super-tile of 2048 pixels):
  1. DVE: 9 bf16 tensor_tensor multiplies prod_t = x_shifted * f_t, one per
     tap, planar. All APs are step-1/4B-aligned so the DVE runs in 2x_1p
     packed mode. Center-column taps (j=1, odd element shift) read from a
     1-element-shifted SBUF copy of x to keep 4B alignment.
  2. PE: 9 accumulating identity matmuls per 512-px PSUM bank sum the taps
     into fp32 PSUM (tensor engine is otherwise idle).
  3. ACT: one activation-copy drains PSUM fp32 -> SBUF bf16.
Inputs/outputs travel as bf16 (halves HBM traffic; rel err ~3e-3), the tap
sum stays fp32 in PSUM. Filter border columns (taps that multiply x-padding
or row-wrapped elements) are zeroed host-side.
"""

import numpy as np
import ml_dtypes

B, C, H, W = 8, 32, 256, 256
K = 3
N_CORES = 8
BAND = H // N_CORES            # 32 rows per core
N_IMG = B * C                  # 256 images
P = 128
GROUPS = N_IMG // P            # 2
RD = 8                         # rows per super-tile
FD = RD * W                    # pixels per super-tile (2048)
SUPERS = BAND // RD            # 4
XW = FD + 2 * W + 2            # x elements per super-tile incl halo+guards
X_FLAT = (BAND + 2) * W + 2    # per-image padded x row storage
BANK = 512                     # PSUM bank capacity in fp32 elements
NTAP = K * K

_CACHE = {}


def _build_module():
    import concourse.bacc as bacc
    import concourse.mybir as mybir
    from concourse.tile import TileContext

    bf16 = mybir.dt.bfloat16
    fp32 = mybir.dt.float32
    mult = mybir.AluOpType.mult

    nc = bacc.Bacc("TRN2", target_bir_lowering=False, debug=False)
    x_d = nc.dram_tensor("x_s", [N_IMG, X_FLAT], bf16,
                         kind="ExternalInput").ap()
    f_d = nc.dram_tensor("f_s", [N_IMG, SUPERS, NTAP, RD, W], bf16,
                         kind="ExternalInput").ap()
    i_d = nc.dram_tensor("ident", [P, P], bf16, kind="ExternalInput").ap()
    o_d = nc.dram_tensor("o_s", [N_IMG, BAND, W], bf16,
                         kind="ExternalOutput").ap()

    with TileContext(nc) as tc:
        with (
            tc.tile_pool(name="ip", bufs=1) as ipool,
            tc.tile_pool(name="xp", bufs=2) as xpool,
            tc.tile_pool(name="xo", bufs=2) as xopool,
            tc.tile_pool(name="fp", bufs=2) as fpool,
            tc.tile_pool(name="pr", bufs=2) as prpool,
            tc.tile_pool(name="op", bufs=2) as opool,
            tc.tile_pool(name="ps", bufs=2, space="PSUM") as pspool,
        ):
            ident = ipool.tile([P, P], bf16)
            nc.sync.dma_start(out=ident[:], in_=i_d[:, :])
            for g in range(GROUPS):
                p0 = g * P
                for s in range(SUPERS):
                    r0 = s * RD
                    xt = xpool.tile([P, XW], bf16, tag="x")
                    nc.gpsimd.dma_start(
                        out=xt[:],
                        in_=x_d[p0:p0 + P, r0 * W: r0 * W + XW])
                    # 1-element-shifted copy: keeps j=1 tap reads 4B-aligned
                    xo = xopool.tile([P, XW - 2], bf16, tag="xo")
                    nc.gpsimd.dma_start(out=xo[:], in_=xt[:, 1:XW - 1])
                    ft = fpool.tile([P, NTAP * FD], bf16, tag="f")
                    nc.sync.dma_start(out=ft[:], in_=f_d[p0:p0 + P, s])
                    pr = prpool.tile([P, NTAP * FD], bf16, tag="pr")
                    for t in range(NTAP):
                        i, j = divmod(t, K)
                        sh = i * W + j
                        if j == 1:
                            src = xo[:, sh - 1: sh - 1 + FD]
                        else:
                            src = xt[:, sh: sh + FD]
                        nc.vector.tensor_tensor(
                            pr[:, t * FD:(t + 1) * FD], src,
                            ft[:, t * FD:(t + 1) * FD], mult)
                    ps = pspool.tile([P, FD], fp32, tag="ps")
                    for b in range(FD // BANK):
                        for t in range(NTAP):
                            off = t * FD + b * BANK
                            nc.tensor.matmul(
                                ps[:, b * BANK:(b + 1) * BANK],
                                ident[:],
                                pr[:, off: off + BANK],
                                start=(t == 0), stop=(t == NTAP - 1))
                    ot = opool.tile([P, FD], bf16, tag="o")
                    nc.scalar.copy(ot[:], ps[:])
                    nc.scalar.dma_start(
                        out=o_d[p0:p0 + P, r0:r0 + RD, :], in_=ot[:])
    nc.compile()
    return nc


def _get_module():
    if "nc" not in _CACHE:
        _CACHE["nc"] = _build_module()
    return _CACHE["nc"]


def _shard_inputs(x, dynamic_filters):
    """Per-core input maps. x: [B,C,H,W] f32, filters: [B,C*9,H,W] f32."""
    bf = ml_dtypes.bfloat16
    xb = x.astype(bf)
    xp = np.pad(xb, ((0, 0), (0, 0), (1, 1), (0, 0)))   # pad rows only
    f6 = dynamic_filters.reshape(B, C, K, K, H, W).astype(bf)
    f6[:, :, :, 0, :, 0] = 0      # j=0 taps multiply x col -1
    f6[:, :, :, 2, :, W - 1] = 0  # j=2 taps multiply x col W
    ftap = f6.reshape(N_IMG, NTAP, H, W)
    ident = np.eye(P, dtype=bf)

    in_maps = []
    for n in range(N_CORES):
        r = n * BAND
        xs = xp[:, :, r:r + BAND + 2, :].reshape(N_IMG, (BAND + 2) * W)
        xs_flat = np.zeros((N_IMG, X_FLAT), bf)
        xs_flat[:, 1:-1] = xs
        fs = ftap[:, :, r:r + BAND].reshape(N_IMG, NTAP, SUPERS, RD, W)
        fs = np.ascontiguousarray(fs.transpose(0, 2, 1, 3, 4))
        in_maps.append({"x_s": xs_flat, "f_s": fs, "ident": ident})
    return in_maps


def kernel(x, dynamic_filters, _trace=False):
    from concourse import bass_utils

    x = np.asarray(x, dtype=np.float32)
    dynamic_filters = np.asarray(dynamic_filters, dtype=np.float32)
    nc = _get_module()
    in_maps = _shard_inputs(x, dynamic_filters)
    res = bass_utils.run_bass_kernel_spmd(
        nc, in_maps, list(range(N_CORES)), trace=_trace)
    out = np.concatenate(
        [np.asarray(res.results[n]["o_s"]).reshape(B, C, BAND, W)
         for n in range(N_CORES)],
        axis=2).astype(np.float32)
    _CACHE["last_exec_time_ns"] = res.exec_time_ns
    if res.instructions_and_trace is not None:
        _CACHE["trace_path"] = res.instructions_and_trace[1]
    return out


# revision 8
# speedup vs baseline: 1.8297x; 1.1787x over previous
"""Dynamic 3x3 per-pixel filter (DynamicFilterLayer2D) on 8 Trainium2 cores.

Reference: out[b,c,h,w] = sum_{i,j in 3x3} xpad[b,c,h+i,w+j] * f[b,c,(3i+j),h,w]

Sharding: H split into 8 bands of 32 rows per core (data parallel, 1-row
halo). Per-core layout: partitions = 128 (b,c) images (2 groups), free dim
= flat pixels.

Compute (per 8-row super-tile of 2048 pixels):
  1. DVE: 9 bf16 tensor_tensor multiplies prod_t = x_shifted * f_t, one per
     tap, planar. All APs are step-1/4B-aligned so the DVE runs in 2x_1p
     packed mode. Center-column taps (j=1, odd element shift) read from a
     1-element-shifted SBUF copy of x to keep 4B alignment.
  2. DVE: two in-place pair-adds fold taps 1,3 into planes 0,2 (bf16 2x),
     balancing DVE vs PE stage time.
  3. PE: 7 accumulating identity matmuls per 512-px PSUM bank sum the
     remaining planes into fp32 PSUM (tensor engine is otherwise idle).
  4. ACT: one activation-copy drains PSUM fp32 -> SBUF bf16.
Inputs/outputs travel as bf16 (halves HBM traffic; rel err ~3e-3), the tap
sum stays fp32 in PSUM. Filter border columns (taps that multiply x-padding
or row-wrapped elements) are zeroed host-side.
"""

import numpy as np
import ml_dtypes

B, C, H, W = 8, 32, 256, 256
K = 3
N_CORES = 8
BAND = H // N_CORES            # 32 rows per core
N_IMG = B * C                  # 256 images
P = 128
GROUPS = N_IMG // P            # 2
RD = 8                         # rows per super-tile
FD = RD * W                    # pixels per super-tile (2048)
SUPERS = BAND // RD            # 4
XW = FD + 2 * W + 2            # x elements per super-tile incl halo+guards
X_FLAT = (BAND + 2) * W + 2    # per-image padded x row storage
BANK = 512                     # PSUM bank capacity in fp32 elements
NTAP = K * K

_CACHE = {}


def _build_module():
    import concourse.bacc as bacc
    import concourse.mybir as mybir
    from concourse.tile import TileContext

    bf16 = mybir.dt.bfloat16
    fp32 = mybir.dt.float32
    mult = mybir.AluOpType.mult

    nc = bacc.Bacc("TRN2", target_bir_lowering=False, debug=False)
    x_d = nc.dram_tensor("x_s", [N_IMG, X_FLAT], bf16,
                         kind="ExternalInput").ap()
    f_d = nc.dram_tensor("f_s", [N_IMG, NTAP, BAND * W], bf16,
                         kind="ExternalInput").ap()
    i_d = nc.dram_tensor("ident", [P, P], bf16, kind="ExternalInput").ap()
    o_d = nc.dram_tensor("o_s", [N_IMG, BAND, W], bf16,
                         kind="ExternalOutput").ap()

    with TileContext(nc) as tc:
        with (
            tc.tile_pool(name="ip", bufs=1) as ipool,
            tc.tile_pool(name="xp", bufs=2) as xpool,
            tc.tile_pool(name="xo", bufs=2) as xopool,
            tc.tile_pool(name="fp", bufs=2) as fpool,
            tc.tile_pool(name="pr", bufs=2) as prpool,
            tc.tile_pool(name="op", bufs=2) as opool,
            tc.tile_pool(name="ps", bufs=2, space="PSUM") as pspool,
        ):
            ident = ipool.tile([P, P], bf16)
            nc.sync.dma_start(out=ident[:], in_=i_d[:, :])
            add = mybir.AluOpType.add
            # tapered super-tile schedule: small first/last shrink pipeline
            # fill and tail
            supers = [(0, 4), (4, 8), (12, 8), (20, 8), (28, 4)]
            # planes summed by PE after the DVE pair-adds (0+=1, 2+=3)
            pe_planes = [0, 2, 4, 5, 6, 7, 8]
            FSPLIT = 6  # f planes 0..5 via sync ring, 6..8 via scalar ring
            for g in range(GROUPS):
                p0 = g * P
                for (r0, rd) in supers:
                    fd = rd * W
                    xw = fd + 2 * W + 2
                    xt = xpool.tile([P, XW], bf16, tag="x")
                    nc.gpsimd.dma_start(
                        out=xt[:, 0:xw],
                        in_=x_d[p0:p0 + P, r0 * W: r0 * W + xw])
                    # 1-element-shifted copy: keeps j=1 tap reads 4B-aligned
                    xo = xopool.tile([P, XW - 2], bf16, tag="xo")
                    nc.gpsimd.dma_start(out=xo[:, 0:xw - 2],
                                        in_=xt[:, 1:xw - 1])
                    ft = fpool.tile([P, NTAP * FD], bf16, tag="f")
                    nc.sync.dma_start(
                        out=ft[:, 0:FSPLIT * fd],
                        in_=f_d[p0:p0 + P, 0:FSPLIT, r0 * W: r0 * W + fd])
                    nc.scalar.dma_start(
                        out=ft[:, FSPLIT * fd:NTAP * fd],
                        in_=f_d[p0:p0 + P, FSPLIT:NTAP, r0 * W: r0 * W + fd])
                    pr = prpool.tile([P, NTAP * FD], bf16, tag="pr")
                    for t in range(NTAP):
                        i, j = divmod(t, K)
                        sh = i * W + j
                        if j == 1:
                            src = xo[:, sh - 1: sh - 1 + fd]
                        else:
                            src = xt[:, sh: sh + fd]
                        nc.vector.tensor_tensor(
                            pr[:, t * fd:(t + 1) * fd], src,
                            ft[:, t * fd:(t + 1) * fd], mult)
                    for (da, db) in ((0, 1), (2, 3)):
                        nc.vector.tensor_tensor(
                            pr[:, da * fd:(da + 1) * fd],
                            pr[:, da * fd:(da + 1) * fd],
                            pr[:, db * fd:(db + 1) * fd], add)
                    ps = pspool.tile([P, FD], fp32, tag="ps")
                    for b in range(fd // BANK):
                        for n, t in enumerate(pe_planes):
                            off = t * fd + b * BANK
                            nc.tensor.matmul(
                                ps[:, b * BANK:(b + 1) * BANK],
                                ident[:],
                                pr[:, off: off + BANK],
                                start=(n == 0), stop=(n == len(pe_planes) - 1))
                    ot = opool.tile([P, FD], bf16, tag="o")
                    nc.scalar.copy(ot[:, 0:fd], ps[:, 0:fd])
                    nc.scalar.dma_start(
                        out=o_d[p0:p0 + P, r0:r0 + rd, :], in_=ot[:, 0:fd])
    nc.compile()
    return nc


def _get_module():
    if "nc" not in _CACHE:
        _CACHE["nc"] = _build_module()
    return _CACHE["nc"]


def _shard_inputs(x, dynamic_filters):
    """Per-core input maps. x: [B,C,H,W] f32, filters: [B,C*9,H,W] f32."""
    bf = ml_dtypes.bfloat16
    xb = x.astype(bf)
    xp = np.pad(xb, ((0, 0), (0, 0), (1, 1), (0, 0)))   # pad rows only
    f6 = dynamic_filters.reshape(B, C, K, K, H, W).astype(bf)
    f6[:, :, :, 0, :, 0] = 0      # j=0 taps multiply x col -1
    f6[:, :, :, 2, :, W - 1] = 0  # j=2 taps multiply x col W
    ftap = f6.reshape(N_IMG, NTAP, H, W)
    ident = np.eye(P, dtype=bf)

    in_maps = []
    for n in range(N_CORES):
        r = n * BAND
        xs = xp[:, :, r:r + BAND + 2, :].reshape(N_IMG, (BAND + 2) * W)
        xs_flat = np.zeros((N_IMG, X_FLAT), bf)
        xs_flat[:, 1:-1] = xs
        fs = np.ascontiguousarray(ftap[:, :, r:r + BAND]).reshape(
            N_IMG, NTAP, BAND * W)
        in_maps.append({"x_s": xs_flat, "f_s": fs, "ident": ident})
    return in_maps


def kernel(x, dynamic_filters, _trace=False):
    from concourse import bass_utils

    x = np.asarray(x, dtype=np.float32)
    dynamic_filters = np.asarray(dynamic_filters, dtype=np.float32)
    nc = _get_module()
    in_maps = _shard_inputs(x, dynamic_filters)
    res = bass_utils.run_bass_kernel_spmd(
        nc, in_maps, list(range(N_CORES)), trace=_trace)
    out = np.concatenate(
        [np.asarray(res.results[n]["o_s"]).reshape(B, C, BAND, W)
         for n in range(N_CORES)],
        axis=2).astype(np.float32)
    _CACHE["last_exec_time_ns"] = res.exec_time_ns
    if res.instructions_and_trace is not None:
        _CACHE["trace_path"] = res.instructions_and_trace[1]
    return out
